# revision 32
# baseline (speedup 1.0000x reference)
"""nn_CPN_67740224192953: full conv pipeline on 8 trn2 cores, minimal I/O.

Device (8 cores, 2 per image = half-image each), per 16-row slab:
  - im2col [27, 22*520] built by 9 strided DMAs from a per-core padded
    canvas in DRAM (1.66 MB/core in, vs 19.7 MB host-im2col before)
  - backbone f = relu(w27.T @ imc), fp16 inputs / f32 PSUM (the host
    re-rank only needs the d map to ~0.05, so fp16 x is safe; f32r's
    un-re-ranked ~1e-3 noise had scrambled the softmax top-k order)
  - 7x7 head for the 3 needed channels [d=s1-s0, ref_x, ref_y] as two
    7-tap shift-accumulate stages:
      Q[(c,dy)] = sum_dx W_dx.T @ f(. + dx)      (K=64, M=21)
      out[c]    = sum_dy S_dy.T @ Q(. + dy*520)  (K=21, M=3)
  - outputs per core, all fp16 (1.05 MB/core, vs 630 MB fp32 partials
    before): d map, and tanh(ref_conv + b_ref) for the two ref channels
Host: global top/bottom 3-row boundary fix; candidate selection from the
  fp16 d map by sigmoid bounds, then exact re-rank + 23-channel head
  (d/loc/fourier) recomputed at the candidates from x patches (restores
  reference top-k order incl. sigmoid-saturation index ties); fourier
  contour synthesis; 4 refinement gather iterations (mirrors reference).
The 8-core PJRT executor (jit + NEFF) is cached at module level; donated
output buffers are created on-device, so a warm kernel() call moves only
~6.7 MB up (fp16 canvas) / ~6.3 MB down through the ~40 MB/s axon tunnel.
"""

import numpy as np

LAST_EXEC_NS = None
LAST_DEVICE_S = None

B, C_IN, H, W = 4, 3, 512, 512
C = 64
ORDER = 5
SAMPLES = 32
N_DET = 512
ITERS = 4
MARGIN = 3.0
K7 = 7
HALF = H // 2         # 256 rows per core
QROWS = 256           # rows per device call (phase); one phase per core
SLAB = 16             # output rows per slab
NSLAB = QROWS // SLAB # 16 slabs per phase
FROWS = SLAB + 6      # f rows per slab (halo 3 top+bottom)
WC = W + 8            # canvas / position-grid width 520
CROWS = QROWS + 10    # canvas rows per phase 266
LPOS = FROWS * WC     # 11440 f/Q positions per slab
LF = 3 + LPOS + 3     # fpad length
OROWS = SLAB * WC     # 8320 out positions per slab
NCH_F = (LPOS + 511) // 512   # 23 chunks
NCH_O = (OROWS + 511) // 512  # 17 chunks

_RUNNER = None        # (sharded_jit, in_names, out_names, out_avals, n_params)


def _build_device_program():
    import concourse.bacc as bacc
    import concourse.mybir as mybir
    from concourse.tile import TileContext

    nc = bacc.Bacc("TRN2", target_bir_lowering=False, num_devices=8)
    f32 = mybir.dt.float32
    f32r = mybir.dt.float32r
    f16 = mybir.dt.float16
    cv_d = nc.dram_tensor("cv", [3, CROWS * WC], f16, kind="ExternalInput")
    w27_d = nc.dram_tensor("w27", [27, C], f16, kind="ExternalInput")
    wdx_d = nc.dram_tensor("wdx", [C, 147], f32, kind="ExternalInput")
    sdy_d = nc.dram_tensor("sdy", [21, 21], f32, kind="ExternalInput")
    z_d = nc.dram_tensor("z", [C, 128], f32, kind="ExternalInput")
    z16_d = nc.dram_tensor("z16", [27, 128], f16, kind="ExternalInput")
    bref_d = nc.dram_tensor("bref", [2, 1], f32, kind="ExternalInput")
    outd_d = nc.dram_tensor("outd", [NSLAB, SLAB, W], f16, kind="ExternalOutput")
    i8 = mybir.dt.int8
    outr_d = nc.dram_tensor("outr", [NSLAB * 2, SLAB, W], i8, kind="ExternalOutput")

    with (
        TileContext(nc) as tc,
        tc.tile_pool(name="wpool", bufs=1) as wpool,
        tc.tile_pool(name="sb", bufs=1) as sb,
        tc.tile_pool(name="ps", bufs=2, space="PSUM") as ps,
        tc.tile_pool(name="ps1", bufs=2, space="PSUM") as ps1,
        tc.tile_pool(name="ps2", bufs=2, space="PSUM") as ps2,
    ):
        # weights: DMA in, then re-copy on DVE so every matmul's weight dep
        # is a DVE semaphore (keeps per-matmul sync-wait count at the limit)
        w27_r = wpool.tile([27, C], f16, tag="w27r")
        wdx_r = wpool.tile([C, 147], f32, tag="wdxr")
        sdy_r = wpool.tile([21, 21], f32, tag="sdyr")
        nc.sync.dma_start(out=w27_r[:], in_=w27_d[:, :])
        nc.sync.dma_start(out=wdx_r[:], in_=wdx_d[:, :])
        nc.sync.dma_start(out=sdy_r[:], in_=sdy_d[:, :])
        bref_r = wpool.tile([2, 1], f32, tag="brefr")
        nc.sync.dma_start(out=bref_r[:], in_=bref_d[:, :])
        w27_t = wpool.tile([27, C], f16, tag="w27")
        wdx_t = wpool.tile([C, 147], f32, tag="wdx")
        sdy_t = wpool.tile([21, 21], f32, tag="sdy")
        bref_t = wpool.tile([2, 1], f32, tag="bref")
        nc.vector.tensor_copy(w27_t[:], w27_r[:])
        nc.vector.tensor_copy(wdx_t[:], wdx_r[:])
        nc.vector.tensor_copy(sdy_t[:], sdy_r[:])
        nc.vector.tensor_copy(bref_t[:], bref_r[:])

        # fpad's flat 3-col pads: written once (relu never touches them;
        # their values only reach discarded edge columns of Q)
        fpad_t = sb.tile([C, LF], f32, tag="fpad")
        nc.sync.dma_start(out=fpad_t[:, 0:3], in_=z_d[:, 0:3])
        nc.sync.dma_start(out=fpad_t[:, 3 + LPOS:], in_=z_d[:, 0:3])

        for s in range(NSLAB):
            # im2col: imc[(dy*3+dx)*3+cin, p] = cv[cin, p + (s*16+dy)*520 + dx]
            imc_t = sb.tile([27, LPOS], f16, tag="imc")
            for j in range(9):
                dy, dx = j // 3, j % 3
                off = (s * SLAB + dy) * WC + dx
                nc.sync.dma_start(out=imc_t[3 * j:3 * j + 3, :],
                                  in_=cv_d[:, off:off + LPOS])
            # zero imc's per-row edge cols (q in [0,3) and [515,520)) so the
            # backbone writes f=relu(0)=0 there — the 7x7 zero-padding of f
            # in the reference
            imc3 = imc_t[:].rearrange("p (r w) -> p r w", w=WC)
            nc.sync.dma_start(
                out=imc3[:, :, 0:3],
                in_=z16_d[:, 0:3 * FROWS].rearrange("p (r w) -> p r w", w=3))
            nc.sync.dma_start(
                out=imc3[:, :, W + 3:WC],
                in_=z16_d[:, 0:5 * FROWS].rearrange("p (r w) -> p r w", w=5))
            # backbone: f = relu(w27.T @ imc), relu on ACT
            for k in range(NCH_F):
                a, b = k * 512, min((k + 1) * 512, LPOS)
                pbb = ps.tile([C, 512], f32, tag="pbb")
                nc.tensor.matmul(out=pbb[:, :b - a],
                                 lhsT=w27_t[:],
                                 rhs=imc_t[:, a:b],
                                 start=True, stop=True)
                nc.scalar.activation(fpad_t[:, 3 + a:3 + b], pbb[:, :b - a],
                                     mybir.ActivationFunctionType.Relu)
            # stage 1: Q[(c*7+dy), p] = sum_dx wdx[:, dx].T @ fpad[p + dx]
            q_t = sb.tile([21, LPOS], f32, tag="q")
            for k in range(NCH_F):
                a, b = k * 512, min((k + 1) * 512, LPOS)
                pq = ps1.tile([21, 512], f32, tag="pq")
                for dx in range(7):
                    nc.tensor.matmul(out=pq[:, :b - a],
                                     lhsT=wdx_t[:, 21 * dx:21 * dx + 21],
                                     rhs=fpad_t[:, a + dx:b + dx],
                                     start=(dx == 0), stop=(dx == 6))
                nc.vector.tensor_copy(q_t[:, a:b], pq[:, :b - a])
            # stage 2: out[c, p] = sum_dy sdy[:, dy].T @ Q[p + dy*520];
            # two PSUM tiles so every PSUM read starts at partition 0:
            # d leaves as fp16 (host re-ranks candidates with exact d),
            # ref channels leave as fp16 tanh(conv+b_ref)
            od_t = sb.tile([1, OROWS], f16, tag="od")
            orf_t = sb.tile([2, OROWS], f16, tag="orf")
            orf8_t = sb.tile([2, OROWS], i8, tag="orf8")
            for k in range(NCH_O):
                a, b = k * 512, min((k + 1) * 512, OROWS)
                po_r = ps2.tile([2, 512], f32, tag="por")
                po_d = ps2.tile([1, 512], f32, tag="pod")
                for dy in range(7):
                    nc.tensor.matmul(out=po_r[:, :b - a],
                                     lhsT=sdy_t[:, 3 * dy:3 * dy + 2],
                                     rhs=q_t[:, a + dy * WC:b + dy * WC],
                                     start=(dy == 0), stop=(dy == 6))
                for dy in range(7):
                    nc.tensor.matmul(out=po_d[:, :b - a],
                                     lhsT=sdy_t[:, 3 * dy + 2:3 * dy + 3],
                                     rhs=q_t[:, a + dy * WC:b + dy * WC],
                                     start=(dy == 0), stop=(dy == 6))
                nc.scalar.activation(orf_t[:, a:b], po_r[:, :b - a],
                                     mybir.ActivationFunctionType.Tanh,
                                     bias=bref_t[:])
                # int8 on the wire: round(127*tanh), dequantized on host
                nc.vector.tensor_scalar_mul(orf8_t[:, a:b], orf_t[:, a:b],
                                            127.0)
                nc.vector.tensor_copy(od_t[:, a:b], po_d[:, :b - a])
            od3 = od_t[:].rearrange("p (t w) -> p t w", w=WC)
            orf3 = orf8_t[:].rearrange("p (t w) -> p t w", w=WC)
            nc.sync.dma_start(out=outd_d[s:s + 1, :, :], in_=od3[:, :, 3:3 + W])
            nc.sync.dma_start(out=outr_d[s * 2:(s + 1) * 2, :, :],
                              in_=orf3[:, :, 3:3 + W])
    nc.finalize()
    return nc


def _get_runner():
    """Build the program + jitted 8-core PJRT executor once per process."""
    global _RUNNER
    if _RUNNER is not None:
        return _RUNNER
    import jax
    from jax.sharding import Mesh, PartitionSpec
    try:
        from jax.experimental.shard_map import shard_map
    except ImportError:
        from jax.shard_map import shard_map
    import concourse.mybir as mybir
    from concourse.bass2jax import (_bass_exec_p, install_neuronx_cc_hook,
                                    partition_id_tensor)

    install_neuronx_cc_hook()
    nc = _build_device_program()
    partition_name = (nc.partition_id_tensor.name
                      if nc.partition_id_tensor else None)
    in_names, out_names, out_avals, zero_shapes = [], [], [], []
    for alloc in nc.m.functions[0].allocations:
        if not isinstance(alloc, mybir.MemoryLocationSet):
            continue
        name = alloc.memorylocations[0].name
        if alloc.kind == "ExternalInput":
            if name != partition_name:
                in_names.append(name)
        elif alloc.kind == "ExternalOutput":
            out_names.append(name)
            shape = tuple(alloc.tensor_shape)
            dtype = mybir.dt.np(alloc.dtype)
            out_avals.append(jax.core.ShapedArray(shape, dtype))
            zero_shapes.append((shape, dtype))
    n_params = len(in_names)
    n_outs = len(out_avals)
    all_names = in_names + out_names
    if partition_name is not None:
        all_names.append(partition_name)
    donate = tuple(range(n_params, n_params + n_outs))

    def _body(*args):
        operands = list(args)
        if partition_name is not None:
            operands.append(partition_id_tensor())
        outs = _bass_exec_p.bind(
            *operands,
            out_avals=tuple(out_avals),
            in_names=tuple(all_names),
            out_names=tuple(out_names),
            lowering_input_output_aliases=(),
            sim_require_finite=True,
            sim_require_nnan=True,
            nc=nc,
        )
        return tuple(outs)

    devices = jax.devices()[:8]
    mesh = Mesh(np.asarray(devices), ("core",))
    in_specs = (PartitionSpec("core"),) * (n_params + n_outs)
    out_specs = (PartitionSpec("core"),) * n_outs
    sharded = jax.jit(
        shard_map(_body, mesh=mesh, in_specs=in_specs, out_specs=out_specs,
                  check_rep=False),
        donate_argnums=donate, keep_unused=True)
    # donated output buffers created on-device (no 12.6 MB H2D of zeros)
    import jax.numpy as jnp
    from jax.sharding import NamedSharding
    zsh = tuple(NamedSharding(mesh, PartitionSpec("core")) for _ in zero_shapes)

    def _zmake():
        return tuple(jnp.zeros((8 * s[0], *s[1:]), d) for s, d in zero_shapes)
    zeros_jit = jax.jit(_zmake, out_shardings=zsh)
    _RUNNER = (sharded, in_names, out_names, out_avals, zero_shapes, zeros_jit)
    return _RUNNER


def _run_device(phase_maps):
    """8-core SPMD execute, one pipelined call per phase; phase-2 upload
    overlaps phase-1 execute/download on the axon tunnel."""
    sharded, in_names, out_names, out_avals, zero_shapes, zeros_jit = _get_runner()
    futs = []
    for in_maps in phase_maps:
        concat_zeros = zeros_jit()   # async on-device donated buffers
        concat_in = [np.concatenate([m[name] for m in in_maps], axis=0)
                     for name in in_names]
        futs.append(sharded(*concat_in, *concat_zeros))
    return [
        [{name: np.asarray(out_arrs[i]).reshape(8, *out_avals[i].shape)[c]
          for i, name in enumerate(out_names)}
         for c in range(8)]
        for out_arrs in futs
    ]


def kernel(x, w_bb, b_bb, w_score, b_score, w_loc, b_loc,
           w_fourier, b_fourier, w_ref, b_ref):
    import time as _time
    x = np.asarray(x, np.float32)
    w_bb = np.asarray(w_bb, np.float32)
    w_score = np.asarray(w_score, np.float32)
    w_loc = np.asarray(w_loc, np.float32)
    w_fourier = np.asarray(w_fourier, np.float32)
    w_ref = np.asarray(w_ref, np.float32)
    b_bb = np.asarray(b_bb, np.float32)

    # ---- weights prep ----
    # w27[(dy*3+dx)*3+cin, cout]
    w27 = np.ascontiguousarray(
        w_bb.transpose(2, 3, 1, 0).reshape(27, C)).astype(np.float16)
    w_d = (w_score[1] - w_score[0]).astype(np.float32)          # [C,7,7]
    whead = np.stack([w_d, w_ref[0], w_ref[1]], 0)              # [3,C,7,7]
    # wdx[ch, dx*21 + c*7 + dy]
    wdx = np.ascontiguousarray(whead.transpose(1, 3, 0, 2).reshape(C, 147))
    # stage-2 selection; output channel order [ref_x, ref_y, d] so the ACT
    # tanh reads PSUM partitions 0:2 (32-aligned base required)
    perm = {0: 2, 1: 0, 2: 1}
    sdy = np.zeros((21, 21), np.float32)
    for c in range(3):
        for dy in range(7):
            sdy[c * 7 + dy, dy * 3 + perm[c]] = 1.0
    # ---- canvases: image rows -4..517, cols -4..515, zero-padded ----
    xgfull = np.zeros((B, 3, H + 10, WC), np.float32)
    xgfull[:, :, 4:4 + H, 4:4 + W] = x
    phase_maps = []
    for ph in range(1):
        in_maps = []
        for core in range(8):
            b, h = core // 2, core % 2
            r0 = h * HALF + ph * QROWS
            cv = xgfull[b, :, r0:r0 + CROWS, :].astype(
                np.float16).reshape(3, CROWS * WC)
            in_maps.append({"cv": cv, "w27": w27, "wdx": wdx, "sdy": sdy,
                            "z": np.zeros((C, 128), np.float32),
                            "z16": np.zeros((27, 128), np.float16),
                            "bref": np.asarray(b_ref, np.float32).reshape(2, 1)})
        phase_maps.append(in_maps)

    # ---- device run ----
    _t0 = _time.time()
    res = _run_device(phase_maps)
    global LAST_EXEC_NS, LAST_DEVICE_S
    LAST_DEVICE_S = _time.time() - _t0
    LAST_EXEC_NS = None

    # ---- host: assemble maps ----
    d_map = np.zeros((B, H, W), np.float32)
    ref_map = np.zeros((B, 2, H, W), np.float32)  # MARGIN*tanh(conv+b_ref)
    for ph in range(1):
        for core in range(8):
            b, h = core // 2, core % 2
            r0 = h * HALF + ph * QROWS
            sl = slice(r0, r0 + QROWS)
            rc = res[ph][core]
            d_map[b, sl] = rc["outd"].astype(np.float32).reshape(QROWS, W)
            orr = rc["outr"].astype(np.float32).reshape(NSLAB, 2, SLAB, W)
            sc = np.float32(MARGIN / 127.0)
            ref_map[b, 0, sl] = sc * orr[:, 0].reshape(QROWS, W)
            ref_map[b, 1, sl] = sc * orr[:, 1].reshape(QROWS, W)

    # ---- host fix of global top/bottom 3 rows (f zero-padding there) ----
    swv = np.lib.stride_tricks.sliding_window_view
    xp = np.pad(x, ((0, 0), (0, 0), (1, 1), (1, 1)))
    for b in range(B):
        for top in (True, False):
            rows = np.arange(0, 6) if top else np.arange(H - 6, H)
            # f rows `rows`: conv3x3 at those image rows
            xwin = swv(xp[b, :, rows[0]:rows[-1] + 3, :], (3, 3),
                       axis=(1, 2))                    # [3, 6, 512, 3, 3]
            fv = np.einsum("crXde,ocde->orX", xwin, w_bb,
                           dtype=np.float32) + b_bb[:, None, None]
            fv = np.maximum(fv, 0.0).astype(np.float32)  # [64, 6, 512]
            # zero-padded f block covering out rows Y (3 rows) needs f rows
            # Y-3..Y+3; rows outside [0,H) are zero
            fz = np.zeros((C, 9, W + 6), np.float32)
            if top:
                fz[:, 3:9, 3:3 + W] = fv                 # f rows 0..5
                yo = np.arange(3)
            else:
                fz[:, 0:6, 3:3 + W] = fv                 # f rows H-6..H-1
                yo = np.arange(H - 3, H)
            fwin = swv(fz, (7, 7), axis=(1, 2))          # [64, 3, 512, 7, 7]
            hmap = np.einsum("kYXab,ckab->cYX", fwin, whead, dtype=np.float32)
            d_map[b, yo] = hmap[0]
            br = np.asarray(b_ref, np.float32)
            ref_map[b, 0, yo] = MARGIN * np.tanh(hmap[1] + br[0])
            ref_map[b, 1, yo] = MARGIN * np.tanh(hmap[2] + br[1])

    bd = np.float32(np.asarray(b_score, np.float32)[1] - np.asarray(b_score, np.float32)[0])
    d_map = d_map + bd

    # ---- candidate selection by approx (fp16) d, exact host re-rank ----
    # device d is fp16 (|err| <= ~0.032 for |d|<64); the exact top-512 and
    # their reference order (f32 sigmoid, saturation ties by index) come
    # from host-recomputed d at the candidates.
    def _fg(dv):
        pos = dv >= 0
        e = np.exp(np.where(pos, -dv, dv).astype(np.float32)).astype(np.float32)
        return np.where(
            pos, (np.float32(1.0) / (np.float32(1.0) + e)).astype(np.float32),
            (e / (np.float32(1.0) + e)).astype(np.float32))

    dd = d_map.reshape(B, H * W).astype(np.float32)
    w23 = np.concatenate([w_d[None], w_loc, w_fourier], 0)   # [23,C,7,7]
    w23f = w23.reshape(23, C * 49)
    b23 = np.concatenate([bd.reshape(1), np.asarray(b_loc, np.float32),
                          np.asarray(b_fourier, np.float32)], 0)
    wbb4 = w_bb.transpose(1, 2, 3, 0)                 # [cin,dy,dx,cout]
    top_idx = np.zeros((B, N_DET), np.int32)
    head22 = np.zeros((B, N_DET, 22), np.float32)
    a_off = np.arange(7)
    for b in range(B):
        # anyone whose fg upper bound beats the 512th-largest lower bound
        # could be in the true top-512 (also covers sigmoid saturation ties)
        eps = np.float32(0.05)
        lo = np.partition(_fg(dd[b] - eps), H * W - N_DET)[H * W - N_DET]
        cand = np.nonzero(_fg(dd[b] + eps) >= lo)[0].astype(np.int64)
        ncand = cand.size
        iy = cand // W
        ix = cand % W
        # f window rows iy-3..iy+3, cols ix-3..ix+3; xg windows via swv
        sw = swv(xgfull[b], (3, 3), axis=(1, 2))
        # sw[c, i, j, dy, dx] = xgfull[c, i+dy, j+dx]; f(Y,X) uses rows Y+3+dy
        rows = iy[:, None, None] + a_off[:, None]
        cols = ix[:, None, None] + a_off[None, :]
        xgwin = sw[:, rows, cols]                     # [3, n, 7, 7, 3, 3]
        fwin = np.einsum("cnabde,cdeo->nabo", xgwin, wbb4,
                         dtype=np.float32) + b_bb[None, None, None, :]
        fwin = np.maximum(fwin, 0.0).astype(np.float32)   # [n,7,7,C]
        # f is zero-padded outside [0,H)x[0,W) for the 7x7 head conv
        wy = iy[:, None] + a_off[None, :] - 3
        wx = ix[:, None] + a_off[None, :] - 3
        mask = (((wy >= 0) & (wy < H))[:, :, None]
                & ((wx >= 0) & (wx < W))[:, None, :])
        fwin *= mask[:, :, :, None]
        vals = fwin.transpose(0, 3, 1, 2).reshape(ncand, C * 49)
        head23 = vals @ w23f.T + b23[None, :]
        fg_exact = _fg(head23[:, 0].astype(np.float32))
        order = np.lexsort((cand, -fg_exact))[:N_DET]
        top_idx[b] = cand[order].astype(np.int32)
        head22[b] = head23[order, 1:23]

    px = (top_idx % W).astype(np.float32)
    py = (top_idx // W).astype(np.float32)

    loc = head22[..., 0:2]
    coef = head22[..., 2:22].reshape(B, N_DET, ORDER, 4)
    cx = (px + loc[..., 0]).astype(np.float32)
    cy = (py + loc[..., 1]).astype(np.float32)

    # ---- fourier contour synthesis ----
    t = np.arange(SAMPLES, dtype=np.float32) / np.float32(SAMPLES)
    kk = np.arange(1, ORDER + 1, dtype=np.float32)
    ang = (np.float32(2.0 * np.pi) * kk[:, None] * t[None, :]).astype(np.float32)
    cos_a = np.cos(ang).astype(np.float32)
    sin_a = np.sin(ang).astype(np.float32)
    xs = (np.einsum("bno,os->bns", coef[..., 0], cos_a, dtype=np.float32)
          + np.einsum("bno,os->bns", coef[..., 1], sin_a, dtype=np.float32)
          + cx[..., None]).astype(np.float32)
    ys = (np.einsum("bno,os->bns", coef[..., 2], cos_a, dtype=np.float32)
          + np.einsum("bno,os->bns", coef[..., 3], sin_a, dtype=np.float32)
          + cy[..., None]).astype(np.float32)
    det = np.stack([xs, ys], -1)

    # ---- refinement iterations ----
    ref_flat = ref_map.reshape(B, 2, H * W)
    for _ in range(ITERS):
        deti = np.round(det)
        xc = np.clip(deti[..., 0], 0, W - 1)
        yc = np.clip(deti[..., 1], 0, H - 1)
        lin = (yc.astype(np.int32) * W + xc.astype(np.int32)).reshape(B, N_DET * SAMPLES)
        rx = np.take_along_axis(ref_flat[:, 0], lin, 1).reshape(B, N_DET, SAMPLES)
        ry = np.take_along_axis(ref_flat[:, 1], lin, 1).reshape(B, N_DET, SAMPLES)
        det = np.stack([(xc + rx).astype(np.float32),
                        (yc + ry).astype(np.float32)], -1)
    return det.astype(np.float32)


# revision 33
# speedup vs baseline: 1.4818x; 1.4818x over previous
"""nn_CPN_67740224192953: full conv pipeline on 8 trn2 cores, minimal I/O.

Device (8 cores, 2 per image = half-image each), per 16-row slab:
  - im2col [27, 22*520] built by 9 strided DMAs from a per-core padded
    canvas in DRAM (1.66 MB/core in, vs 19.7 MB host-im2col before)
  - backbone f = relu(w27.T @ imc), fp16 inputs / f32 PSUM (the host
    re-rank only needs the d map to ~0.05, so fp16 x is safe; f32r's
    un-re-ranked ~1e-3 noise had scrambled the softmax top-k order)
  - 7x7 head for the 3 needed channels [d=s1-s0, ref_x, ref_y] as two
    7-tap shift-accumulate stages:
      Q[(c,dy)] = sum_dx W_dx.T @ f(. + dx)      (K=64, M=21)
      out[c]    = sum_dy S_dy.T @ Q(. + dy*520)  (K=21, M=3)
  - outputs per core, all fp16 (1.05 MB/core, vs 630 MB fp32 partials
    before): d map, and tanh(ref_conv + b_ref) for the two ref channels
Host: global top/bottom 3-row boundary fix; candidate selection from the
  fp16 d map by sigmoid bounds, then exact re-rank + 23-channel head
  (d/loc/fourier) recomputed at the candidates from x patches (restores
  reference top-k order incl. sigmoid-saturation index ties); fourier
  contour synthesis; 4 refinement gather iterations (mirrors reference).
The 8-core PJRT executor (jit + NEFF) is cached at module level; donated
output buffers are created on-device, so a warm kernel() call moves only
~6.7 MB up (fp16 canvas) / ~6.3 MB down through the ~40 MB/s axon tunnel.
"""

import numpy as np

LAST_EXEC_NS = None
LAST_DEVICE_S = None

B, C_IN, H, W = 4, 3, 512, 512
C = 64
ORDER = 5
SAMPLES = 32
N_DET = 512
ITERS = 4
MARGIN = 3.0
K7 = 7
HALF = H // 2         # 256 rows per core
QROWS = 256           # rows per device call (phase); one phase per core
SLAB = 16             # output rows per slab
NSLAB = QROWS // SLAB # 16 slabs per phase
FROWS = SLAB + 6      # f rows per slab (halo 3 top+bottom)
WC = W + 8            # canvas / position-grid width 520
CROWS = QROWS + 10    # canvas rows per phase 266
LPOS = FROWS * WC     # 11440 f/Q positions per slab
LF = 3 + LPOS + 3     # fpad length
OROWS = SLAB * WC     # 8320 out positions per slab
NCH_F = (LPOS + 511) // 512   # 23 chunks
NCH_O = (OROWS + 511) // 512  # 17 chunks

_RUNNER = None        # (sharded_jit, in_names, out_names, out_avals, n_params)


def _build_device_program():
    import concourse.bacc as bacc
    import concourse.mybir as mybir
    from concourse.tile import TileContext

    nc = bacc.Bacc("TRN2", target_bir_lowering=False, num_devices=8)
    f32 = mybir.dt.float32
    f32r = mybir.dt.float32r
    f16 = mybir.dt.float16
    cv_d = nc.dram_tensor("cv", [3, CROWS * WC], f16, kind="ExternalInput")
    w27_d = nc.dram_tensor("w27", [27, C], f16, kind="ExternalInput")
    wdx_d = nc.dram_tensor("wdx", [C, 147], f32, kind="ExternalInput")
    sdy_d = nc.dram_tensor("sdy", [21, 21], f32, kind="ExternalInput")
    z_d = nc.dram_tensor("z", [C, 128], f32, kind="ExternalInput")
    z16_d = nc.dram_tensor("z16", [27, 128], f16, kind="ExternalInput")
    bref_d = nc.dram_tensor("bref", [2, 1], f32, kind="ExternalInput")
    outd_d = nc.dram_tensor("outd", [NSLAB, SLAB, W], f16, kind="ExternalOutput")
    outr_d = nc.dram_tensor("outr", [NSLAB * 2, SLAB, W], f16, kind="ExternalOutput")

    with (
        TileContext(nc) as tc,
        tc.tile_pool(name="wpool", bufs=1) as wpool,
        tc.tile_pool(name="sb", bufs=1) as sb,
        tc.tile_pool(name="ps", bufs=2, space="PSUM") as ps,
        tc.tile_pool(name="ps1", bufs=2, space="PSUM") as ps1,
        tc.tile_pool(name="ps2", bufs=2, space="PSUM") as ps2,
    ):
        # weights: DMA in, then re-copy on DVE so every matmul's weight dep
        # is a DVE semaphore (keeps per-matmul sync-wait count at the limit)
        w27_r = wpool.tile([27, C], f16, tag="w27r")
        wdx_r = wpool.tile([C, 147], f32, tag="wdxr")
        sdy_r = wpool.tile([21, 21], f32, tag="sdyr")
        nc.sync.dma_start(out=w27_r[:], in_=w27_d[:, :])
        nc.sync.dma_start(out=wdx_r[:], in_=wdx_d[:, :])
        nc.sync.dma_start(out=sdy_r[:], in_=sdy_d[:, :])
        bref_r = wpool.tile([2, 1], f32, tag="brefr")
        nc.sync.dma_start(out=bref_r[:], in_=bref_d[:, :])
        w27_t = wpool.tile([27, C], f16, tag="w27")
        wdx_t = wpool.tile([C, 147], f32, tag="wdx")
        sdy_t = wpool.tile([21, 21], f32, tag="sdy")
        bref_t = wpool.tile([2, 1], f32, tag="bref")
        nc.vector.tensor_copy(w27_t[:], w27_r[:])
        nc.vector.tensor_copy(wdx_t[:], wdx_r[:])
        nc.vector.tensor_copy(sdy_t[:], sdy_r[:])
        nc.vector.tensor_copy(bref_t[:], bref_r[:])

        # fpad's flat 3-col pads: written once (relu never touches them;
        # their values only reach discarded edge columns of Q)
        fpad_t = sb.tile([C, LF], f32, tag="fpad")
        nc.sync.dma_start(out=fpad_t[:, 0:3], in_=z_d[:, 0:3])
        nc.sync.dma_start(out=fpad_t[:, 3 + LPOS:], in_=z_d[:, 0:3])

        for s in range(NSLAB):
            # im2col: imc[(dy*3+dx)*3+cin, p] = cv[cin, p + (s*16+dy)*520 + dx]
            imc_t = sb.tile([27, LPOS], f16, tag="imc")
            for j in range(9):
                dy, dx = j // 3, j % 3
                off = (s * SLAB + dy) * WC + dx
                nc.sync.dma_start(out=imc_t[3 * j:3 * j + 3, :],
                                  in_=cv_d[:, off:off + LPOS])
            # zero imc's per-row edge cols (q in [0,3) and [515,520)) so the
            # backbone writes f=relu(0)=0 there — the 7x7 zero-padding of f
            # in the reference
            imc3 = imc_t[:].rearrange("p (r w) -> p r w", w=WC)
            nc.sync.dma_start(
                out=imc3[:, :, 0:3],
                in_=z16_d[:, 0:3 * FROWS].rearrange("p (r w) -> p r w", w=3))
            nc.sync.dma_start(
                out=imc3[:, :, W + 3:WC],
                in_=z16_d[:, 0:5 * FROWS].rearrange("p (r w) -> p r w", w=5))
            # backbone: f = relu(w27.T @ imc), relu on ACT
            for k in range(NCH_F):
                a, b = k * 512, min((k + 1) * 512, LPOS)
                pbb = ps.tile([C, 512], f32, tag="pbb")
                nc.tensor.matmul(out=pbb[:, :b - a],
                                 lhsT=w27_t[:],
                                 rhs=imc_t[:, a:b],
                                 start=True, stop=True)
                nc.scalar.activation(fpad_t[:, 3 + a:3 + b], pbb[:, :b - a],
                                     mybir.ActivationFunctionType.Relu)
            # stage 1: Q[(c*7+dy), p] = sum_dx wdx[:, dx].T @ fpad[p + dx]
            q_t = sb.tile([21, LPOS], f32, tag="q")
            for k in range(NCH_F):
                a, b = k * 512, min((k + 1) * 512, LPOS)
                pq = ps1.tile([21, 512], f32, tag="pq")
                for dx in range(7):
                    nc.tensor.matmul(out=pq[:, :b - a],
                                     lhsT=wdx_t[:, 21 * dx:21 * dx + 21],
                                     rhs=fpad_t[:, a + dx:b + dx],
                                     start=(dx == 0), stop=(dx == 6))
                nc.vector.tensor_copy(q_t[:, a:b], pq[:, :b - a])
            # stage 2: out[c, p] = sum_dy sdy[:, dy].T @ Q[p + dy*520];
            # two PSUM tiles so every PSUM read starts at partition 0:
            # d leaves as fp16 (host re-ranks candidates with exact d),
            # ref channels leave as fp16 tanh(conv+b_ref)
            od_t = sb.tile([1, OROWS], f16, tag="od")
            orf_t = sb.tile([2, OROWS], f16, tag="orf")
            for k in range(NCH_O):
                a, b = k * 512, min((k + 1) * 512, OROWS)
                po_r = ps2.tile([2, 512], f32, tag="por")
                po_d = ps2.tile([1, 512], f32, tag="pod")
                for dy in range(7):
                    nc.tensor.matmul(out=po_r[:, :b - a],
                                     lhsT=sdy_t[:, 3 * dy:3 * dy + 2],
                                     rhs=q_t[:, a + dy * WC:b + dy * WC],
                                     start=(dy == 0), stop=(dy == 6))
                for dy in range(7):
                    nc.tensor.matmul(out=po_d[:, :b - a],
                                     lhsT=sdy_t[:, 3 * dy + 2:3 * dy + 3],
                                     rhs=q_t[:, a + dy * WC:b + dy * WC],
                                     start=(dy == 0), stop=(dy == 6))
                nc.scalar.activation(orf_t[:, a:b], po_r[:, :b - a],
                                     mybir.ActivationFunctionType.Tanh,
                                     bias=bref_t[:])
                nc.vector.tensor_copy(od_t[:, a:b], po_d[:, :b - a])
            od3 = od_t[:].rearrange("p (t w) -> p t w", w=WC)
            orf3 = orf_t[:].rearrange("p (t w) -> p t w", w=WC)
            nc.sync.dma_start(out=outd_d[s:s + 1, :, :], in_=od3[:, :, 3:3 + W])
            nc.sync.dma_start(out=outr_d[s * 2:(s + 1) * 2, :, :],
                              in_=orf3[:, :, 3:3 + W])
    nc.finalize()
    return nc


def _get_runner():
    """Build the program + jitted 8-core PJRT executor once per process."""
    global _RUNNER
    if _RUNNER is not None:
        return _RUNNER
    import jax
    from jax.sharding import Mesh, PartitionSpec
    try:
        from jax.experimental.shard_map import shard_map
    except ImportError:
        from jax.shard_map import shard_map
    import concourse.mybir as mybir
    from concourse.bass2jax import (_bass_exec_p, install_neuronx_cc_hook,
                                    partition_id_tensor)

    install_neuronx_cc_hook()
    nc = _build_device_program()
    partition_name = (nc.partition_id_tensor.name
                      if nc.partition_id_tensor else None)
    in_names, out_names, out_avals, zero_shapes = [], [], [], []
    for alloc in nc.m.functions[0].allocations:
        if not isinstance(alloc, mybir.MemoryLocationSet):
            continue
        name = alloc.memorylocations[0].name
        if alloc.kind == "ExternalInput":
            if name != partition_name:
                in_names.append(name)
        elif alloc.kind == "ExternalOutput":
            out_names.append(name)
            shape = tuple(alloc.tensor_shape)
            dtype = mybir.dt.np(alloc.dtype)
            out_avals.append(jax.core.ShapedArray(shape, dtype))
            zero_shapes.append((shape, dtype))
    n_params = len(in_names)
    n_outs = len(out_avals)
    all_names = in_names + out_names
    if partition_name is not None:
        all_names.append(partition_name)
    donate = tuple(range(n_params, n_params + n_outs))

    def _body(*args):
        operands = list(args)
        if partition_name is not None:
            operands.append(partition_id_tensor())
        outs = _bass_exec_p.bind(
            *operands,
            out_avals=tuple(out_avals),
            in_names=tuple(all_names),
            out_names=tuple(out_names),
            lowering_input_output_aliases=(),
            sim_require_finite=True,
            sim_require_nnan=True,
            nc=nc,
        )
        return tuple(outs)

    devices = jax.devices()[:8]
    mesh = Mesh(np.asarray(devices), ("core",))
    in_specs = (PartitionSpec("core"),) * (n_params + n_outs)
    out_specs = (PartitionSpec("core"),) * n_outs
    sharded = jax.jit(
        shard_map(_body, mesh=mesh, in_specs=in_specs, out_specs=out_specs,
                  check_rep=False),
        donate_argnums=donate, keep_unused=True)
    # donated output buffers created on-device (no 12.6 MB H2D of zeros)
    import jax.numpy as jnp
    from jax.sharding import NamedSharding
    zsh = tuple(NamedSharding(mesh, PartitionSpec("core")) for _ in zero_shapes)

    def _zmake():
        return tuple(jnp.zeros((8 * s[0], *s[1:]), d) for s, d in zero_shapes)
    zeros_jit = jax.jit(_zmake, out_shardings=zsh)
    _RUNNER = (sharded, in_names, out_names, out_avals, zero_shapes, zeros_jit)
    return _RUNNER


def _run_device(phase_maps):
    """8-core SPMD execute, one pipelined call per phase; phase-2 upload
    overlaps phase-1 execute/download on the axon tunnel."""
    sharded, in_names, out_names, out_avals, zero_shapes, zeros_jit = _get_runner()
    futs = []
    for in_maps in phase_maps:
        concat_zeros = zeros_jit()   # async on-device donated buffers
        concat_in = [np.concatenate([m[name] for m in in_maps], axis=0)
                     for name in in_names]
        futs.append(sharded(*concat_in, *concat_zeros))
    return [
        [{name: np.asarray(out_arrs[i]).reshape(8, *out_avals[i].shape)[c]
          for i, name in enumerate(out_names)}
         for c in range(8)]
        for out_arrs in futs
    ]


def kernel(x, w_bb, b_bb, w_score, b_score, w_loc, b_loc,
           w_fourier, b_fourier, w_ref, b_ref):
    import time as _time
    x = np.asarray(x, np.float32)
    w_bb = np.asarray(w_bb, np.float32)
    w_score = np.asarray(w_score, np.float32)
    w_loc = np.asarray(w_loc, np.float32)
    w_fourier = np.asarray(w_fourier, np.float32)
    w_ref = np.asarray(w_ref, np.float32)
    b_bb = np.asarray(b_bb, np.float32)

    # ---- weights prep ----
    # w27[(dy*3+dx)*3+cin, cout]
    w27 = np.ascontiguousarray(
        w_bb.transpose(2, 3, 1, 0).reshape(27, C)).astype(np.float16)
    w_d = (w_score[1] - w_score[0]).astype(np.float32)          # [C,7,7]
    whead = np.stack([w_d, w_ref[0], w_ref[1]], 0)              # [3,C,7,7]
    # wdx[ch, dx*21 + c*7 + dy]
    wdx = np.ascontiguousarray(whead.transpose(1, 3, 0, 2).reshape(C, 147))
    # stage-2 selection; output channel order [ref_x, ref_y, d] so the ACT
    # tanh reads PSUM partitions 0:2 (32-aligned base required)
    perm = {0: 2, 1: 0, 2: 1}
    sdy = np.zeros((21, 21), np.float32)
    for c in range(3):
        for dy in range(7):
            sdy[c * 7 + dy, dy * 3 + perm[c]] = 1.0
    # ---- canvases: image rows -4..517, cols -4..515, zero-padded ----
    xgfull = np.zeros((B, 3, H + 10, WC), np.float32)
    xgfull[:, :, 4:4 + H, 4:4 + W] = x
    phase_maps = []
    for ph in range(1):
        in_maps = []
        for core in range(8):
            b, h = core // 2, core % 2
            r0 = h * HALF + ph * QROWS
            cv = xgfull[b, :, r0:r0 + CROWS, :].astype(
                np.float16).reshape(3, CROWS * WC)
            in_maps.append({"cv": cv, "w27": w27, "wdx": wdx, "sdy": sdy,
                            "z": np.zeros((C, 128), np.float32),
                            "z16": np.zeros((27, 128), np.float16),
                            "bref": np.asarray(b_ref, np.float32).reshape(2, 1)})
        phase_maps.append(in_maps)

    # ---- device run ----
    _t0 = _time.time()
    res = _run_device(phase_maps)
    global LAST_EXEC_NS, LAST_DEVICE_S
    LAST_DEVICE_S = _time.time() - _t0
    LAST_EXEC_NS = None

    # ---- host: assemble maps ----
    d_map = np.zeros((B, H, W), np.float32)
    ref_map = np.zeros((B, 2, H, W), np.float32)  # MARGIN*tanh(conv+b_ref)
    for ph in range(1):
        for core in range(8):
            b, h = core // 2, core % 2
            r0 = h * HALF + ph * QROWS
            sl = slice(r0, r0 + QROWS)
            rc = res[ph][core]
            d_map[b, sl] = rc["outd"].astype(np.float32).reshape(QROWS, W)
            orr = rc["outr"].astype(np.float32).reshape(NSLAB, 2, SLAB, W)
            ref_map[b, 0, sl] = MARGIN * orr[:, 0].reshape(QROWS, W)
            ref_map[b, 1, sl] = MARGIN * orr[:, 1].reshape(QROWS, W)

    # ---- host fix of global top/bottom 3 rows (f zero-padding there) ----
    swv = np.lib.stride_tricks.sliding_window_view
    xp = np.pad(x, ((0, 0), (0, 0), (1, 1), (1, 1)))
    for b in range(B):
        for top in (True, False):
            rows = np.arange(0, 6) if top else np.arange(H - 6, H)
            # f rows `rows`: conv3x3 at those image rows
            xwin = swv(xp[b, :, rows[0]:rows[-1] + 3, :], (3, 3),
                       axis=(1, 2))                    # [3, 6, 512, 3, 3]
            fv = np.einsum("crXde,ocde->orX", xwin, w_bb,
                           dtype=np.float32) + b_bb[:, None, None]
            fv = np.maximum(fv, 0.0).astype(np.float32)  # [64, 6, 512]
            # zero-padded f block covering out rows Y (3 rows) needs f rows
            # Y-3..Y+3; rows outside [0,H) are zero
            fz = np.zeros((C, 9, W + 6), np.float32)
            if top:
                fz[:, 3:9, 3:3 + W] = fv                 # f rows 0..5
                yo = np.arange(3)
            else:
                fz[:, 0:6, 3:3 + W] = fv                 # f rows H-6..H-1
                yo = np.arange(H - 3, H)
            fwin = swv(fz, (7, 7), axis=(1, 2))          # [64, 3, 512, 7, 7]
            hmap = np.einsum("kYXab,ckab->cYX", fwin, whead, dtype=np.float32)
            d_map[b, yo] = hmap[0]
            br = np.asarray(b_ref, np.float32)
            ref_map[b, 0, yo] = MARGIN * np.tanh(hmap[1] + br[0])
            ref_map[b, 1, yo] = MARGIN * np.tanh(hmap[2] + br[1])

    bd = np.float32(np.asarray(b_score, np.float32)[1] - np.asarray(b_score, np.float32)[0])
    d_map = d_map + bd

    # ---- candidate selection by approx (fp16) d, exact host re-rank ----
    # device d is fp16 (|err| <= ~0.032 for |d|<64); the exact top-512 and
    # their reference order (f32 sigmoid, saturation ties by index) come
    # from host-recomputed d at the candidates.
    def _fg(dv):
        pos = dv >= 0
        e = np.exp(np.where(pos, -dv, dv).astype(np.float32)).astype(np.float32)
        return np.where(
            pos, (np.float32(1.0) / (np.float32(1.0) + e)).astype(np.float32),
            (e / (np.float32(1.0) + e)).astype(np.float32))

    dd = d_map.reshape(B, H * W).astype(np.float32)
    w23 = np.concatenate([w_d[None], w_loc, w_fourier], 0)   # [23,C,7,7]
    w23f = w23.reshape(23, C * 49)
    b23 = np.concatenate([bd.reshape(1), np.asarray(b_loc, np.float32),
                          np.asarray(b_fourier, np.float32)], 0)
    wbb4 = w_bb.transpose(1, 2, 3, 0)                 # [cin,dy,dx,cout]
    top_idx = np.zeros((B, N_DET), np.int32)
    head22 = np.zeros((B, N_DET, 22), np.float32)
    a_off = np.arange(7)
    for b in range(B):
        # anyone whose fg upper bound beats the 512th-largest lower bound
        # could be in the true top-512 (also covers sigmoid saturation ties)
        eps = np.float32(0.05)
        lo = np.partition(_fg(dd[b] - eps), H * W - N_DET)[H * W - N_DET]
        cand = np.nonzero(_fg(dd[b] + eps) >= lo)[0].astype(np.int64)
        ncand = cand.size
        iy = cand // W
        ix = cand % W
        # f window rows iy-3..iy+3, cols ix-3..ix+3; xg windows via swv
        sw = swv(xgfull[b], (3, 3), axis=(1, 2))
        # sw[c, i, j, dy, dx] = xgfull[c, i+dy, j+dx]; f(Y,X) uses rows Y+3+dy
        rows = iy[:, None, None] + a_off[:, None]
        cols = ix[:, None, None] + a_off[None, :]
        xgwin = sw[:, rows, cols]                     # [3, n, 7, 7, 3, 3]
        fwin = np.einsum("cnabde,cdeo->nabo", xgwin, wbb4,
                         dtype=np.float32) + b_bb[None, None, None, :]
        fwin = np.maximum(fwin, 0.0).astype(np.float32)   # [n,7,7,C]
        # f is zero-padded outside [0,H)x[0,W) for the 7x7 head conv
        wy = iy[:, None] + a_off[None, :] - 3
        wx = ix[:, None] + a_off[None, :] - 3
        mask = (((wy >= 0) & (wy < H))[:, :, None]
                & ((wx >= 0) & (wx < W))[:, None, :])
        fwin *= mask[:, :, :, None]
        vals = fwin.transpose(0, 3, 1, 2).reshape(ncand, C * 49)
        head23 = vals @ w23f.T + b23[None, :]
        fg_exact = _fg(head23[:, 0].astype(np.float32))
        order = np.lexsort((cand, -fg_exact))[:N_DET]
        top_idx[b] = cand[order].astype(np.int32)
        head22[b] = head23[order, 1:23]

    px = (top_idx % W).astype(np.float32)
    py = (top_idx // W).astype(np.float32)

    loc = head22[..., 0:2]
    coef = head22[..., 2:22].reshape(B, N_DET, ORDER, 4)
    cx = (px + loc[..., 0]).astype(np.float32)
    cy = (py + loc[..., 1]).astype(np.float32)

    # ---- fourier contour synthesis ----
    t = np.arange(SAMPLES, dtype=np.float32) / np.float32(SAMPLES)
    kk = np.arange(1, ORDER + 1, dtype=np.float32)
    ang = (np.float32(2.0 * np.pi) * kk[:, None] * t[None, :]).astype(np.float32)
    cos_a = np.cos(ang).astype(np.float32)
    sin_a = np.sin(ang).astype(np.float32)
    xs = (np.einsum("bno,os->bns", coef[..., 0], cos_a, dtype=np.float32)
          + np.einsum("bno,os->bns", coef[..., 1], sin_a, dtype=np.float32)
          + cx[..., None]).astype(np.float32)
    ys = (np.einsum("bno,os->bns", coef[..., 2], cos_a, dtype=np.float32)
          + np.einsum("bno,os->bns", coef[..., 3], sin_a, dtype=np.float32)
          + cy[..., None]).astype(np.float32)
    det = np.stack([xs, ys], -1)

    # ---- refinement iterations ----
    ref_flat = ref_map.reshape(B, 2, H * W)
    for _ in range(ITERS):
        deti = np.round(det)
        xc = np.clip(deti[..., 0], 0, W - 1)
        yc = np.clip(deti[..., 1], 0, H - 1)
        lin = (yc.astype(np.int32) * W + xc.astype(np.int32)).reshape(B, N_DET * SAMPLES)
        rx = np.take_along_axis(ref_flat[:, 0], lin, 1).reshape(B, N_DET, SAMPLES)
        ry = np.take_along_axis(ref_flat[:, 1], lin, 1).reshape(B, N_DET, SAMPLES)
        det = np.stack([(xc + rx).astype(np.float32),
                        (yc + ry).astype(np.float32)], -1)
    return det.astype(np.float32)


# revision 34
# speedup vs baseline: 1.7766x; 1.1990x over previous
"""nn_CPN_67740224192953: full conv pipeline on 8 trn2 cores, minimal I/O.

Device (8 cores, 2 per image = half-image each), per 16-row slab:
  - im2col [27, 22*520] built by 9 strided DMAs from a per-core padded
    canvas in DRAM (1.66 MB/core in, vs 19.7 MB host-im2col before)
  - backbone f = relu(w27.T @ imc), fp16 inputs / f32 PSUM (the host
    re-rank only needs the d map to ~0.05, so fp16 x is safe; f32r's
    un-re-ranked ~1e-3 noise had scrambled the softmax top-k order)
  - 7x7 head for the 3 needed channels [d=s1-s0, ref_x, ref_y] as two
    7-tap shift-accumulate stages:
      Q[(c,dy)] = sum_dx W_dx.T @ f(. + dx)      (K=64, M=21)
      out[c]    = sum_dy S_dy.T @ Q(. + dy*520)  (K=21, M=3)
  - outputs per core, all fp16 (1.05 MB/core, vs 630 MB fp32 partials
    before): d map, and tanh(ref_conv + b_ref) for the two ref channels
Host: global top/bottom 3-row boundary fix; candidate selection from the
  fp16 d map by sigmoid bounds, then exact re-rank + 23-channel head
  (d/loc/fourier) recomputed at the candidates from x patches (restores
  reference top-k order incl. sigmoid-saturation index ties); fourier
  contour synthesis; 4 refinement gather iterations (mirrors reference).
The 8-core PJRT executor (jit + NEFF) is cached at module level; donated
output buffers are created on-device, so a warm kernel() call moves only
~6.7 MB up (fp16 canvas) / ~6.3 MB down through the ~40 MB/s axon tunnel.
"""

import numpy as np

LAST_EXEC_NS = None
LAST_DEVICE_S = None

B, C_IN, H, W = 4, 3, 512, 512
C = 64
ORDER = 5
SAMPLES = 32
N_DET = 512
ITERS = 4
MARGIN = 3.0
K7 = 7
HALF = H // 2         # 256 rows per core
QROWS = 256           # rows per device call (phase); one phase per core
SLAB = 16             # output rows per slab
NSLAB = QROWS // SLAB # 16 slabs per phase
FROWS = SLAB + 6      # f rows per slab (halo 3 top+bottom)
WC = W + 8            # canvas / position-grid width 520
CROWS = QROWS + 10    # canvas rows per phase 266
LPOS = FROWS * WC     # 11440 f/Q positions per slab
LF = 3 + LPOS + 3     # fpad length
OROWS = SLAB * WC     # 8320 out positions per slab
NCH_F = (LPOS + 511) // 512   # 23 chunks
NCH_O = (OROWS + 511) // 512  # 17 chunks

_RUNNER = None        # (sharded_jit, in_names, out_names, out_avals, n_params)


def _build_device_program():
    import concourse.bacc as bacc
    import concourse.mybir as mybir
    from concourse.tile import TileContext

    nc = bacc.Bacc("TRN2", target_bir_lowering=False, num_devices=8)
    f32 = mybir.dt.float32
    f32r = mybir.dt.float32r
    f16 = mybir.dt.float16
    cv_d = nc.dram_tensor("cv", [3, CROWS * WC], f16, kind="ExternalInput")
    # packed params: pk32 = [wdx | sdy | zeros | bref], pk16 = [w27 | zeros]
    pk32_d = nc.dram_tensor("pk32", [C, 172], f32, kind="ExternalInput")
    pk16_d = nc.dram_tensor("pk16", [27, 174], f16, kind="ExternalInput")
    out_d = nc.dram_tensor("out", [NSLAB * 3, SLAB, W], f16, kind="ExternalOutput")

    with (
        TileContext(nc) as tc,
        tc.tile_pool(name="wpool", bufs=1) as wpool,
        tc.tile_pool(name="sb", bufs=1) as sb,
        tc.tile_pool(name="ps", bufs=2, space="PSUM") as ps,
        tc.tile_pool(name="ps1", bufs=2, space="PSUM") as ps1,
        tc.tile_pool(name="ps2", bufs=2, space="PSUM") as ps2,
    ):
        # weights: DMA in, then re-copy on DVE so every matmul's weight dep
        # is a DVE semaphore (keeps per-matmul sync-wait count at the limit)
        w27_r = wpool.tile([27, C], f16, tag="w27r")
        wdx_r = wpool.tile([C, 147], f32, tag="wdxr")
        sdy_r = wpool.tile([21, 21], f32, tag="sdyr")
        nc.sync.dma_start(out=w27_r[:], in_=pk16_d[:, 0:C])
        nc.sync.dma_start(out=wdx_r[:], in_=pk32_d[:, 0:147])
        nc.sync.dma_start(out=sdy_r[:], in_=pk32_d[0:21, 147:168])
        bref_r = wpool.tile([2, 1], f32, tag="brefr")
        nc.sync.dma_start(out=bref_r[:], in_=pk32_d[0:2, 171:172])
        w27_t = wpool.tile([27, C], f16, tag="w27")
        wdx_t = wpool.tile([C, 147], f32, tag="wdx")
        sdy_t = wpool.tile([21, 21], f32, tag="sdy")
        bref_t = wpool.tile([2, 1], f32, tag="bref")
        nc.vector.tensor_copy(w27_t[:], w27_r[:])
        nc.vector.tensor_copy(wdx_t[:], wdx_r[:])
        nc.vector.tensor_copy(sdy_t[:], sdy_r[:])
        nc.vector.tensor_copy(bref_t[:], bref_r[:])

        # fpad's flat 3-col pads: written once (relu never touches them;
        # their values only reach discarded edge columns of Q)
        fpad_t = sb.tile([C, LF], f32, tag="fpad")
        nc.sync.dma_start(out=fpad_t[:, 0:3], in_=pk32_d[:, 168:171])
        nc.sync.dma_start(out=fpad_t[:, 3 + LPOS:], in_=pk32_d[:, 168:171])

        for s in range(NSLAB):
            # im2col: imc[(dy*3+dx)*3+cin, p] = cv[cin, p + (s*16+dy)*520 + dx]
            imc_t = sb.tile([27, LPOS], f16, tag="imc")
            for j in range(9):
                dy, dx = j // 3, j % 3
                off = (s * SLAB + dy) * WC + dx
                nc.sync.dma_start(out=imc_t[3 * j:3 * j + 3, :],
                                  in_=cv_d[:, off:off + LPOS])
            # zero imc's per-row edge cols (q in [0,3) and [515,520)) so the
            # backbone writes f=relu(0)=0 there — the 7x7 zero-padding of f
            # in the reference
            imc3 = imc_t[:].rearrange("p (r w) -> p r w", w=WC)
            nc.sync.dma_start(
                out=imc3[:, :, 0:3],
                in_=pk16_d[:, C:C + 3 * FROWS].rearrange("p (r w) -> p r w", w=3))
            nc.sync.dma_start(
                out=imc3[:, :, W + 3:WC],
                in_=pk16_d[:, C:C + 5 * FROWS].rearrange("p (r w) -> p r w", w=5))
            # backbone: f = relu(w27.T @ imc), relu on ACT
            for k in range(NCH_F):
                a, b = k * 512, min((k + 1) * 512, LPOS)
                pbb = ps.tile([C, 512], f32, tag="pbb")
                nc.tensor.matmul(out=pbb[:, :b - a],
                                 lhsT=w27_t[:],
                                 rhs=imc_t[:, a:b],
                                 start=True, stop=True)
                nc.scalar.activation(fpad_t[:, 3 + a:3 + b], pbb[:, :b - a],
                                     mybir.ActivationFunctionType.Relu)
            # stage 1: Q[(c*7+dy), p] = sum_dx wdx[:, dx].T @ fpad[p + dx]
            q_t = sb.tile([21, LPOS], f32, tag="q")
            for k in range(NCH_F):
                a, b = k * 512, min((k + 1) * 512, LPOS)
                pq = ps1.tile([21, 512], f32, tag="pq")
                for dx in range(7):
                    nc.tensor.matmul(out=pq[:, :b - a],
                                     lhsT=wdx_t[:, 21 * dx:21 * dx + 21],
                                     rhs=fpad_t[:, a + dx:b + dx],
                                     start=(dx == 0), stop=(dx == 6))
                nc.vector.tensor_copy(q_t[:, a:b], pq[:, :b - a])
            # stage 2: out[c, p] = sum_dy sdy[:, dy].T @ Q[p + dy*520];
            # two PSUM tiles so every PSUM read starts at partition 0:
            # d leaves as fp16 (host re-ranks candidates with exact d),
            # ref channels leave as fp16 tanh(conv+b_ref)
            od_t = sb.tile([1, OROWS], f16, tag="od")
            orf_t = sb.tile([2, OROWS], f16, tag="orf")
            for k in range(NCH_O):
                a, b = k * 512, min((k + 1) * 512, OROWS)
                po_r = ps2.tile([2, 512], f32, tag="por")
                po_d = ps2.tile([1, 512], f32, tag="pod")
                for dy in range(7):
                    nc.tensor.matmul(out=po_r[:, :b - a],
                                     lhsT=sdy_t[:, 3 * dy:3 * dy + 2],
                                     rhs=q_t[:, a + dy * WC:b + dy * WC],
                                     start=(dy == 0), stop=(dy == 6))
                for dy in range(7):
                    nc.tensor.matmul(out=po_d[:, :b - a],
                                     lhsT=sdy_t[:, 3 * dy + 2:3 * dy + 3],
                                     rhs=q_t[:, a + dy * WC:b + dy * WC],
                                     start=(dy == 0), stop=(dy == 6))
                nc.scalar.activation(orf_t[:, a:b], po_r[:, :b - a],
                                     mybir.ActivationFunctionType.Tanh,
                                     bias=bref_t[:])
                nc.vector.tensor_copy(od_t[:, a:b], po_d[:, :b - a])
            od3 = od_t[:].rearrange("p (t w) -> p t w", w=WC)
            orf3 = orf_t[:].rearrange("p (t w) -> p t w", w=WC)
            nc.sync.dma_start(out=out_d[s * 3 + 2:s * 3 + 3, :, :],
                              in_=od3[:, :, 3:3 + W])
            nc.sync.dma_start(out=out_d[s * 3:s * 3 + 2, :, :],
                              in_=orf3[:, :, 3:3 + W])
    nc.finalize()
    return nc


def _get_runner():
    """Build the program + jitted 8-core PJRT executor once per process."""
    global _RUNNER
    if _RUNNER is not None:
        return _RUNNER
    import jax
    from jax.sharding import Mesh, PartitionSpec
    try:
        from jax.experimental.shard_map import shard_map
    except ImportError:
        from jax.shard_map import shard_map
    import concourse.mybir as mybir
    from concourse.bass2jax import (_bass_exec_p, install_neuronx_cc_hook,
                                    partition_id_tensor)

    install_neuronx_cc_hook()
    nc = _build_device_program()
    partition_name = (nc.partition_id_tensor.name
                      if nc.partition_id_tensor else None)
    in_names, out_names, out_avals, zero_shapes = [], [], [], []
    for alloc in nc.m.functions[0].allocations:
        if not isinstance(alloc, mybir.MemoryLocationSet):
            continue
        name = alloc.memorylocations[0].name
        if alloc.kind == "ExternalInput":
            if name != partition_name:
                in_names.append(name)
        elif alloc.kind == "ExternalOutput":
            out_names.append(name)
            shape = tuple(alloc.tensor_shape)
            dtype = mybir.dt.np(alloc.dtype)
            out_avals.append(jax.core.ShapedArray(shape, dtype))
            zero_shapes.append((shape, dtype))
    n_params = len(in_names)
    n_outs = len(out_avals)
    all_names = in_names + out_names
    if partition_name is not None:
        all_names.append(partition_name)
    donate = tuple(range(n_params, n_params + n_outs))

    def _body(*args):
        operands = list(args)
        if partition_name is not None:
            operands.append(partition_id_tensor())
        outs = _bass_exec_p.bind(
            *operands,
            out_avals=tuple(out_avals),
            in_names=tuple(all_names),
            out_names=tuple(out_names),
            lowering_input_output_aliases=(),
            sim_require_finite=True,
            sim_require_nnan=True,
            nc=nc,
        )
        return tuple(outs)

    devices = jax.devices()[:8]
    mesh = Mesh(np.asarray(devices), ("core",))
    in_specs = (PartitionSpec("core"),) * (n_params + n_outs)
    out_specs = (PartitionSpec("core"),) * n_outs
    sharded = jax.jit(
        shard_map(_body, mesh=mesh, in_specs=in_specs, out_specs=out_specs,
                  check_rep=False),
        donate_argnums=donate, keep_unused=True)
    # donated output buffers created on-device (no 12.6 MB H2D of zeros)
    import jax.numpy as jnp
    from jax.sharding import NamedSharding
    zsh = tuple(NamedSharding(mesh, PartitionSpec("core")) for _ in zero_shapes)

    def _zmake():
        return tuple(jnp.zeros((8 * s[0], *s[1:]), d) for s, d in zero_shapes)
    zeros_jit = jax.jit(_zmake, out_shardings=zsh)
    _RUNNER = (sharded, in_names, out_names, out_avals, zero_shapes, zeros_jit)
    return _RUNNER


def _run_device(phase_maps):
    """8-core SPMD execute, one pipelined call per phase; phase-2 upload
    overlaps phase-1 execute/download on the axon tunnel."""
    sharded, in_names, out_names, out_avals, zero_shapes, zeros_jit = _get_runner()
    futs = []
    for in_maps in phase_maps:
        concat_zeros = zeros_jit()   # async on-device donated buffers
        concat_in = [np.concatenate([m[name] for m in in_maps], axis=0)
                     for name in in_names]
        futs.append(sharded(*concat_in, *concat_zeros))
    return [
        [{name: np.asarray(out_arrs[i]).reshape(8, *out_avals[i].shape)[c]
          for i, name in enumerate(out_names)}
         for c in range(8)]
        for out_arrs in futs
    ]


def kernel(x, w_bb, b_bb, w_score, b_score, w_loc, b_loc,
           w_fourier, b_fourier, w_ref, b_ref):
    import time as _time
    x = np.asarray(x, np.float32)
    w_bb = np.asarray(w_bb, np.float32)
    w_score = np.asarray(w_score, np.float32)
    w_loc = np.asarray(w_loc, np.float32)
    w_fourier = np.asarray(w_fourier, np.float32)
    w_ref = np.asarray(w_ref, np.float32)
    b_bb = np.asarray(b_bb, np.float32)

    # ---- weights prep ----
    # w27[(dy*3+dx)*3+cin, cout]
    w27 = np.ascontiguousarray(
        w_bb.transpose(2, 3, 1, 0).reshape(27, C)).astype(np.float16)
    w_d = (w_score[1] - w_score[0]).astype(np.float32)          # [C,7,7]
    whead = np.stack([w_d, w_ref[0], w_ref[1]], 0)              # [3,C,7,7]
    # wdx[ch, dx*21 + c*7 + dy]
    wdx = np.ascontiguousarray(whead.transpose(1, 3, 0, 2).reshape(C, 147))
    # stage-2 selection; output channel order [ref_x, ref_y, d] so the ACT
    # tanh reads PSUM partitions 0:2 (32-aligned base required)
    perm = {0: 2, 1: 0, 2: 1}
    sdy = np.zeros((21, 21), np.float32)
    for c in range(3):
        for dy in range(7):
            sdy[c * 7 + dy, dy * 3 + perm[c]] = 1.0
    # ---- canvases: image rows -4..517, cols -4..515, zero-padded ----
    pk32 = np.zeros((C, 172), np.float32)
    pk32[:, 0:147] = wdx
    pk32[0:21, 147:168] = sdy
    pk32[0:2, 171:172] = np.asarray(b_ref, np.float32).reshape(2, 1)
    pk16 = np.zeros((27, 174), np.float16)
    pk16[:, 0:C] = w27
    xgfull = np.zeros((B, 3, H + 10, WC), np.float32)
    xgfull[:, :, 4:4 + H, 4:4 + W] = x
    phase_maps = []
    for ph in range(1):
        in_maps = []
        for core in range(8):
            b, h = core // 2, core % 2
            r0 = h * HALF + ph * QROWS
            cv = xgfull[b, :, r0:r0 + CROWS, :].astype(
                np.float16).reshape(3, CROWS * WC)
            in_maps.append({"cv": cv, "pk32": pk32, "pk16": pk16})
        phase_maps.append(in_maps)

    # ---- device run ----
    _t0 = _time.time()
    res = _run_device(phase_maps)
    global LAST_EXEC_NS, LAST_DEVICE_S
    LAST_DEVICE_S = _time.time() - _t0
    LAST_EXEC_NS = None

    # ---- host: assemble maps ----
    d_map = np.zeros((B, H, W), np.float32)
    ref_map = np.zeros((B, 2, H, W), np.float32)  # MARGIN*tanh(conv+b_ref)
    for ph in range(1):
        for core in range(8):
            b, h = core // 2, core % 2
            r0 = h * HALF + ph * QROWS
            sl = slice(r0, r0 + QROWS)
            oo = res[ph][core]["out"].astype(np.float32).reshape(
                NSLAB, 3, SLAB, W)
            d_map[b, sl] = oo[:, 2].reshape(QROWS, W)
            ref_map[b, 0, sl] = MARGIN * oo[:, 0].reshape(QROWS, W)
            ref_map[b, 1, sl] = MARGIN * oo[:, 1].reshape(QROWS, W)

    # ---- host fix of global top/bottom 3 rows (f zero-padding there) ----
    swv = np.lib.stride_tricks.sliding_window_view
    xp = np.pad(x, ((0, 0), (0, 0), (1, 1), (1, 1)))
    for b in range(B):
        for top in (True, False):
            rows = np.arange(0, 6) if top else np.arange(H - 6, H)
            # f rows `rows`: conv3x3 at those image rows
            xwin = swv(xp[b, :, rows[0]:rows[-1] + 3, :], (3, 3),
                       axis=(1, 2))                    # [3, 6, 512, 3, 3]
            fv = np.einsum("crXde,ocde->orX", xwin, w_bb,
                           dtype=np.float32) + b_bb[:, None, None]
            fv = np.maximum(fv, 0.0).astype(np.float32)  # [64, 6, 512]
            # zero-padded f block covering out rows Y (3 rows) needs f rows
            # Y-3..Y+3; rows outside [0,H) are zero
            fz = np.zeros((C, 9, W + 6), np.float32)
            if top:
                fz[:, 3:9, 3:3 + W] = fv                 # f rows 0..5
                yo = np.arange(3)
            else:
                fz[:, 0:6, 3:3 + W] = fv                 # f rows H-6..H-1
                yo = np.arange(H - 3, H)
            fwin = swv(fz, (7, 7), axis=(1, 2))          # [64, 3, 512, 7, 7]
            hmap = np.einsum("kYXab,ckab->cYX", fwin, whead, dtype=np.float32)
            d_map[b, yo] = hmap[0]
            br = np.asarray(b_ref, np.float32)
            ref_map[b, 0, yo] = MARGIN * np.tanh(hmap[1] + br[0])
            ref_map[b, 1, yo] = MARGIN * np.tanh(hmap[2] + br[1])

    bd = np.float32(np.asarray(b_score, np.float32)[1] - np.asarray(b_score, np.float32)[0])
    d_map = d_map + bd

    # ---- candidate selection by approx (fp16) d, exact host re-rank ----
    # device d is fp16 (|err| <= ~0.032 for |d|<64); the exact top-512 and
    # their reference order (f32 sigmoid, saturation ties by index) come
    # from host-recomputed d at the candidates.
    def _fg(dv):
        pos = dv >= 0
        e = np.exp(np.where(pos, -dv, dv).astype(np.float32)).astype(np.float32)
        return np.where(
            pos, (np.float32(1.0) / (np.float32(1.0) + e)).astype(np.float32),
            (e / (np.float32(1.0) + e)).astype(np.float32))

    dd = d_map.reshape(B, H * W).astype(np.float32)
    w23 = np.concatenate([w_d[None], w_loc, w_fourier], 0)   # [23,C,7,7]
    w23f = w23.reshape(23, C * 49)
    b23 = np.concatenate([bd.reshape(1), np.asarray(b_loc, np.float32),
                          np.asarray(b_fourier, np.float32)], 0)
    wbb4 = w_bb.transpose(1, 2, 3, 0)                 # [cin,dy,dx,cout]
    top_idx = np.zeros((B, N_DET), np.int32)
    head22 = np.zeros((B, N_DET, 22), np.float32)
    a_off = np.arange(7)
    for b in range(B):
        # anyone whose fg upper bound beats the 512th-largest lower bound
        # could be in the true top-512 (also covers sigmoid saturation ties)
        eps = np.float32(0.05)
        lo = np.partition(_fg(dd[b] - eps), H * W - N_DET)[H * W - N_DET]
        cand = np.nonzero(_fg(dd[b] + eps) >= lo)[0].astype(np.int64)
        ncand = cand.size
        iy = cand // W
        ix = cand % W
        # f window rows iy-3..iy+3, cols ix-3..ix+3; xg windows via swv
        sw = swv(xgfull[b], (3, 3), axis=(1, 2))
        # sw[c, i, j, dy, dx] = xgfull[c, i+dy, j+dx]; f(Y,X) uses rows Y+3+dy
        rows = iy[:, None, None] + a_off[:, None]
        cols = ix[:, None, None] + a_off[None, :]
        xgwin = sw[:, rows, cols]                     # [3, n, 7, 7, 3, 3]
        fwin = np.einsum("cnabde,cdeo->nabo", xgwin, wbb4,
                         dtype=np.float32) + b_bb[None, None, None, :]
        fwin = np.maximum(fwin, 0.0).astype(np.float32)   # [n,7,7,C]
        # f is zero-padded outside [0,H)x[0,W) for the 7x7 head conv
        wy = iy[:, None] + a_off[None, :] - 3
        wx = ix[:, None] + a_off[None, :] - 3
        mask = (((wy >= 0) & (wy < H))[:, :, None]
                & ((wx >= 0) & (wx < W))[:, None, :])
        fwin *= mask[:, :, :, None]
        vals = fwin.transpose(0, 3, 1, 2).reshape(ncand, C * 49)
        head23 = vals @ w23f.T + b23[None, :]
        fg_exact = _fg(head23[:, 0].astype(np.float32))
        order = np.lexsort((cand, -fg_exact))[:N_DET]
        top_idx[b] = cand[order].astype(np.int32)
        head22[b] = head23[order, 1:23]

    px = (top_idx % W).astype(np.float32)
    py = (top_idx // W).astype(np.float32)

    loc = head22[..., 0:2]
    coef = head22[..., 2:22].reshape(B, N_DET, ORDER, 4)
    cx = (px + loc[..., 0]).astype(np.float32)
    cy = (py + loc[..., 1]).astype(np.float32)

    # ---- fourier contour synthesis ----
    t = np.arange(SAMPLES, dtype=np.float32) / np.float32(SAMPLES)
    kk = np.arange(1, ORDER + 1, dtype=np.float32)
    ang = (np.float32(2.0 * np.pi) * kk[:, None] * t[None, :]).astype(np.float32)
    cos_a = np.cos(ang).astype(np.float32)
    sin_a = np.sin(ang).astype(np.float32)
    xs = (np.einsum("bno,os->bns", coef[..., 0], cos_a, dtype=np.float32)
          + np.einsum("bno,os->bns", coef[..., 1], sin_a, dtype=np.float32)
          + cx[..., None]).astype(np.float32)
    ys = (np.einsum("bno,os->bns", coef[..., 2], cos_a, dtype=np.float32)
          + np.einsum("bno,os->bns", coef[..., 3], sin_a, dtype=np.float32)
          + cy[..., None]).astype(np.float32)
    det = np.stack([xs, ys], -1)

    # ---- refinement iterations ----
    ref_flat = ref_map.reshape(B, 2, H * W)
    for _ in range(ITERS):
        deti = np.round(det)
        xc = np.clip(deti[..., 0], 0, W - 1)
        yc = np.clip(deti[..., 1], 0, H - 1)
        lin = (yc.astype(np.int32) * W + xc.astype(np.int32)).reshape(B, N_DET * SAMPLES)
        rx = np.take_along_axis(ref_flat[:, 0], lin, 1).reshape(B, N_DET, SAMPLES)
        ry = np.take_along_axis(ref_flat[:, 1], lin, 1).reshape(B, N_DET, SAMPLES)
        det = np.stack([(xc + rx).astype(np.float32),
                        (yc + ry).astype(np.float32)], -1)
    return det.astype(np.float32)
